# revision 49
# baseline (speedup 1.0000x reference)
"""GCN forward (4x GCNConv + linear head) on 8 Trainium2 NeuronCores.

Sharding: nodes are partitioned across the 8 cores (2048 rows each).

Transpose-free layout cycle: the aggregation contracts over edge slots
(gathered rows stationary, scatter matrix moving) and lands feature-major
in PSUM; the dense GEMM contracts over features (aggregated tile
stationary, natural-layout weights moving) and lands node-major, which is
exactly the layout the AllGather and the next layer's row gather need.
Layers 1-3 aggregate first (A_hat h) W; layer 4 runs its GEMM first
(aggregating at width 2048 instead of 4096); the head GEMM consumes the
layer-4 aggregation (feature-major) directly.

Weights stream once per layer (W3 twice, once per node half); W4 is held
resident in a phase-scoped pool that reuses SBUF released by the earlier
phases.  Bias is applied with one extra matmul per accumulation group
(lhsT = first-row-ones, rhs = bias row), so node-major outputs need no
per-free-element bias pass.  The per-tile gather slots are sorted by
source (tile position, core) so one slot order serves all three h_full
layouts; the normalization coefficients and edge->tile assignment are
precomputed on the host.

The AllGathers are chunked unevenly per tensor (B_H2/B_H3/B_M4): each AG
op costs a ~25-35us floor on the serial chain AND blocks the in-order
gpsimd stream for its whole duration, so h2/h3 use a small HEAD chunk
(chain starts right after the first GEMM tile) and m4 small TAIL chunks
(the last block lands right after the last GEMM tile, and the head
phase's first gather groups are hoisted ahead of the blocking tail AG).
"""

import os

import numpy as np
import ml_dtypes

try:  # persistent compilation cache: skip walrus recompile across processes
    import jax
    jax.config.update("jax_compilation_cache_dir",
                      os.path.expanduser("~/.cache/jax_gcn_kernel"))
    jax.config.update("jax_persistent_cache_min_compile_time_secs", 10)
except Exception:
    pass

import concourse.bass as bass
import concourse.mybir as mybir
import concourse.tile as tile
from concourse import bacc
from concourse.bass_utils import run_bass_kernel_spmd

N = 16384
E = 65536
NCORES = 8
NL = N // NCORES            # 2048 nodes per core
NT = NL // 128              # 16 dst tiles per core
DIMS = [512, 1024, 2048, 4096, 2048]
C = 1000
CPAD = 1024                 # output classes padded to a multiple of 128

# AllGather chunking, in tiles (sum = NT), per AG'd tensor.  Each AG op
# costs a ~22us ncfw floor on the serial chain, so few chunks win; a small
# HEAD chunk starts the chain right after the first GEMM tile (h2, h3 --
# their chains are CC-bound, so the end time = start + total); a small TAIL
# chunk only pays off when the chain is GEMM-paced with slack (m4), landing
# the last block right after the last GEMM tile.
B_H2 = (1, 7, 8)
B_H3 = (2, 4, 4, 4, 2)
B_M4 = (4, 4, 4, 2, 2)


def _layout(blocks):
    """(SBT tile prefix, FULLBASE row base, SBIDX tile-prefix->block)."""
    sbt = tuple(int(s) for s in np.cumsum((0,) + blocks))
    fullbase = tuple(s * 128 * NCORES for s in sbt)
    sbidx = {sbt[k + 1]: k for k in range(len(blocks))}
    return sbt, fullbase, sbidx


LAY2 = _layout(B_H2)
LAY3 = _layout(B_H3)
LAY4 = _layout(B_M4)

BF = mybir.dt.bfloat16
F32 = mybir.dt.float32
I16 = mybir.dt.int16
RELU = mybir.ActivationFunctionType.Relu
COPY = mybir.ActivationFunctionType.Copy
NPBF = ml_dtypes.bfloat16

_CACHE = {}


# ----------------------------------------------------------------------------
# Device program
# ----------------------------------------------------------------------------

def _build(chunks, cnt_t, dep2, dep3, dep4):
    nc = bacc.Bacc("TRN2", target_bir_lowering=False, num_devices=NCORES)

    # ---- kernel I/O ----
    xg_d = nc.dram_tensor("xg", [128, NT, chunks, DIMS[0]], BF,
                          kind="ExternalInput")
    idx2_d = nc.dram_tensor("idx2", [128, NT, chunks * 8], I16,
                            kind="ExternalInput")
    idx3_d = nc.dram_tensor("idx3", [128, NT, chunks * 8], I16,
                            kind="ExternalInput")
    idx4_d = nc.dram_tensor("idx4", [128, NT, chunks * 8], I16,
                            kind="ExternalInput")
    s_d = nc.dram_tensor("smat", [128, NT, chunks, 128], BF, kind="ExternalInput")
    ones_d = nc.dram_tensor("onesrow", [128, 512], BF, kind="ExternalInput")
    w1_d = nc.dram_tensor("w1n", [128, 4, 1024], BF, kind="ExternalInput")
    w2_d = nc.dram_tensor("w2n", [128, 8, 2048], BF, kind="ExternalInput")
    w3_d = nc.dram_tensor("w3b", [128, 32, 16, 128], BF, kind="ExternalInput")
    w4_d = nc.dram_tensor("w4n", [128, 32, 2048], BF, kind="ExternalInput")
    wo_d = nc.dram_tensor("won", [128, 16, CPAD], BF, kind="ExternalInput")
    brow1_d = nc.dram_tensor("brow1", [128, 1024], BF, kind="ExternalInput")
    brow2_d = nc.dram_tensor("brow2", [128, 2048], BF, kind="ExternalInput")
    browo_d = nc.dram_tensor("browo", [128, CPAD], BF, kind="ExternalInput")
    b3c_d = nc.dram_tensor("b3c", [128, 32], F32, kind="ExternalInput")
    b4c_d = nc.dram_tensor("b4c", [128, 16], F32, kind="ExternalInput")
    out_d = nc.dram_tensor("out_nm", [NL, CPAD], F32, kind="ExternalOutput")
    dbg_d = dbg3_d = dbg4_d = None
    if os.environ.get("GCN_DEBUG_H2"):
        dbg_d = nc.dram_tensor("dbg_h2", [N, DIMS[1]], BF,
                               kind="ExternalOutput")
        dbg3_d = nc.dram_tensor("dbg_h3", [N, DIMS[2]], BF,
                                kind="ExternalOutput")
        dbg4_d = nc.dram_tensor("dbg_m4", [N, DIMS[4]], BF,
                                kind="ExternalOutput")

    # ---- internal DRAM ----
    h2_nm = nc.dram_tensor("h2_nm", [NL, DIMS[1]], BF)
    h2_full = nc.dram_tensor("h2_full", [N, DIMS[1]], BF, addr_space="Shared")
    h3_nm = nc.dram_tensor("h3_nm", [NL, DIMS[2]], BF)
    h3_full = nc.dram_tensor("h3_full", [N, DIMS[2]], BF, addr_space="Shared")
    h4T = nc.dram_tensor("h4T", [NT, 128, 32, 128], BF)   # [t][feat_p][kb][node]
    m4_nm = nc.dram_tensor("m4_nm", [NL, DIMS[4]], BF)
    m4_full = nc.dram_tensor("m4_full", [N, DIMS[4]], BF, addr_space="Shared")

    rg = [list(range(NCORES))]

    with tile.TileContext(nc) as tc:
        with (
            tc.tile_pool(name="const", bufs=1) as p_const,
            tc.tile_pool(name="brow", bufs=1) as p_brow,
            tc.tile_pool(name="bcol", bufs=2) as p_bcol,
            tc.tile_pool(name="ht", bufs=2) as p_ht,
            tc.tile_pool(name="w3s", bufs=2) as p_w3s,
            tc.tile_pool(name="aggps", bufs=3, space="PSUM") as p_agg,
            tc.tile_pool(name="warmps", bufs=1, space="PSUM") as p_warm,
            tc.tile_pool(name="gemmps", bufs=2, space="PSUM") as p_gemm,
        ):
            # ---- constants ----
            idx_sb = {}
            for nm_, d_ in (("idx2", idx2_d), ("idx3", idx3_d),
                            ("idx4", idx4_d)):
                idx_sb[nm_] = p_const.tile([128, NT, chunks * 8], I16,
                                           name=nm_)
                nc.sync.dma_start(out=idx_sb[nm_][:], in_=d_[:])
            s_sb = p_const.tile([128, NT, chunks, 128], BF)
            nc.sync.dma_start(out=s_sb[:], in_=s_d[:])
            ones_sb = p_const.tile([128, 512], BF)
            nc.sync.dma_start(out=ones_sb[:], in_=ones_d[:])

            # src graph layouts: (idx tile, per-chunk dep, FULLBASE)
            src2 = (idx_sb["idx2"], dep2, _layout(B_H2)[1])
            src3 = (idx_sb["idx3"], dep3, _layout(B_H3)[1])
            src4 = (idx_sb["idx4"], dep4, _layout(B_M4)[1])

            def mk_groups(dept, t):
                nct = cnt_t[t]
                groups = []
                c0 = 0
                for c in range(1, nct + 1):
                    if c == nct or dept[t][c] != dept[t][c0]:
                        groups.append((c0, c, dept[t][c0]))
                        c0 = c
                return groups

            def emit_gathers(gath, src_full, fa, src_lay, t, groups):
                isb, dept, fullbase = src_lay
                for (c0, c1, dep) in groups:
                    ng = c1 - c0
                    rows = fullbase[dep + 1]
                    nc.gpsimd.dma_gather(
                        gath[:, c0:c1, :], src_full.ap()[0:rows, :],
                        isb[:, t, c0 * 8:c1 * 8],
                        ng * 128, ng * 128, fa,
                    )

            def hoist_early(src_full, fa, src_lay, nblk, tiles=(0, 1, 2)):
                """Pre-stage the next layer's first tiles' gather groups for
                all blocks but the tail, ahead of the blocking tail AG."""
                out = {}
                for t in tiles:
                    gath = p_gath.tile([128, chunks, fa], BF, tag="gath",
                                       name=f"hoist{t}")
                    gs = mk_groups(src_lay[1], t)
                    early = [g for g in gs if g[2] < nblk - 1]
                    late = [g for g in gs if g[2] >= nblk - 1]
                    emit_gathers(gath, src_full, fa, src_lay, t, early)
                    out[t] = (gath, late)
                return out

            def agg_tile(src_full, fa, t, tt, aggT, src_lay=None, bcol=None,
                         warm=False, pre=None):
                """Aggregate one dst tile t at width fa into aggT columns
                [tt*128,(tt+1)*128), feature-major bf16."""
                if True:
                    nct = cnt_t[t]
                    if pre is not None:
                        gath, late = pre
                        emit_gathers(gath, src_full, fa, src_lay, t, late)
                    else:
                        gath = p_gath.tile([128, chunks, fa], BF, tag="gath")
                        if src_full is None:
                            # layer 1: rows pre-gathered on the host; plain
                            # contiguous HWDGE loads, no Q7 descriptor work
                            nc.sync.dma_start(out=gath[:, 0:nct, :],
                                              in_=xg_d[:, t, 0:nct, :])
                        else:
                            emit_gathers(gath, src_full, fa, src_lay, t,
                                         mk_groups(src_lay[1], t))
                    for g in range(fa // 512):
                        aps = p_agg.tile([128, 4, 128], F32, tag="aggps")
                        for q in range(4):
                            fb = g * 4 + q
                            for c in range(nct):
                                nc.tensor.matmul(
                                    out=aps[:, q, :],
                                    lhsT=gath[:, c, fb * 128:(fb + 1) * 128],
                                    rhs=s_sb[:, t, c, :],
                                    start=(c == 0), stop=(c == nct - 1),
                                )
                        if bcol is None:
                            nc.vector.tensor_copy(
                                out=aggT[:, g * 4:(g + 1) * 4,
                                         tt * 128:(tt + 1) * 128],
                                in_=aps[:],
                            )
                        else:
                            for q in range(4):
                                fb = g * 4 + q
                                nc.vector.tensor_scalar(
                                    out=aggT[:, fb, tt * 128:(tt + 1) * 128],
                                    in0=aps[:, q, :],
                                    scalar1=bcol[:, fb:fb + 1], scalar2=0.0,
                                    op0=mybir.AluOpType.add,
                                    op1=mybir.AluOpType.max,
                                )
                    if warm:
                        # keep the PE HAM activity window busy through the
                        # gather-bound stretches so matmuls stay warm
                        wp = p_warm.tile([128, 512], F32, tag="warmps")
                        for _ in range(6):
                            nc.tensor.matmul(out=wp[:], lhsT=ones_sb[:, 0:128],
                                             rhs=ones_sb[:], start=True,
                                             stop=True)

            def agg_half(src_full, fa, half, aggT, src_lay=None, bcol=None,
                         warm=False):
                for tt in range(8):
                    agg_tile(src_full, fa, half * 8 + tt, tt, aggT,
                             src_lay=src_lay, bcol=bcol, warm=warm)

            def gemm_tile(aggT, half, tt, fa, fo, w_sb, brow, relu,
                          out_dram, out_f32=None):
                nkb = fa // 128
                nhc = max(fo // 1024, 1)
                fc = min(fo, 1024)
                if True:
                    row0 = (half * 8 + tt) * 128
                    for hc in range(nhc):
                        gps = p_gemm.tile([128, fc], F32, tag="gemmps")
                        for kb in range(nkb):
                            for cc in range(fc // 512):
                                nc.tensor.matmul(
                                    out=gps[:, cc * 512:(cc + 1) * 512],
                                    lhsT=aggT[:, kb, tt * 128:(tt + 1) * 128],
                                    rhs=w_sb[:, kb,
                                             hc * fc + cc * 512:
                                             hc * fc + (cc + 1) * 512],
                                    start=(kb == 0), stop=False,
                                )
                        for cc in range(fc // 512):
                            nc.tensor.matmul(
                                out=gps[:, cc * 512:(cc + 1) * 512],
                                lhsT=ones_sb[:, 0:128],
                                rhs=brow[:, hc * fc + cc * 512:
                                         hc * fc + (cc + 1) * 512],
                                start=False, stop=True,
                            )
                        if out_f32 is not None:
                            of = out_f32.tile([128, fc], F32, tag="outf")
                            nc.vector.tensor_copy(out=of[:], in_=gps[:])
                            nc.sync.dma_start(
                                out=out_dram.ap()[row0:row0 + 128,
                                                  hc * fc:(hc + 1) * fc],
                                in_=of[:],
                            )
                        else:
                            ht = p_ht.tile([128, fc], BF, tag="ht")
                            if relu:
                                nc.scalar.activation(out=ht[:], in_=gps[:],
                                                     func=RELU)
                            else:
                                nc.vector.tensor_copy(out=ht[:], in_=gps[:])
                            nc.sync.dma_start(
                                out=out_dram.ap()[row0:row0 + 128,
                                                  hc * fc:(hc + 1) * fc],
                                in_=ht[:],
                            )

            def gemm_fm3(aggT, half, b3c, mbs=range(32)):
                """Layer-3 GEMM, feature-major out: h4T = relu(W3.T @ aggT + b3).

                lhsT = W3 blocks (streamed once per half), rhs = aggT."""
                for mb in mbs:
                    w3t = p_w3s.tile([128, 16, 128], BF, tag="w3s")
                    nc.sync.dma_start(out=w3t[:], in_=w3_d[:, mb, :, :])
                    gps = p_gemm.tile([128, 1024], F32, tag="gemmps")
                    for kb in range(16):
                        for cc in range(2):
                            nc.tensor.matmul(
                                out=gps[:, cc * 512:(cc + 1) * 512],
                                lhsT=w3t[:, kb, :],
                                rhs=aggT[:, kb, cc * 512:(cc + 1) * 512],
                                start=(kb == 0), stop=(kb == 15),
                            )
                    ht = p_ht.tile([128, 1024], BF, tag="ht")
                    nc.scalar.activation(out=ht[:], in_=gps[:], func=RELU,
                                         bias=b3c[:, mb:mb + 1])
                    nc.sync.dma_start(
                        out=h4T.ap()[half * 8:(half + 1) * 8, :, mb, :]
                            .rearrange("t p n -> p t n"),
                        in_=ht[:].rearrange("p (t n) -> p t n", t=8),
                    )

            def allgather_rows(nm, full, k, lay):
                sbt, fullbase, _ = lay
                nc.gpsimd.collective_compute(
                    "AllGather", mybir.AluOpType.bypass, replica_groups=rg,
                    ins=[nm.ap()[sbt[k] * 128:sbt[k + 1] * 128, :].opt()],
                    outs=[full.ap()[fullbase[k]:fullbase[k + 1], :].opt()],
                )

            # ================= layers 1-3 =================
            with (
                tc.tile_pool(name="wres", bufs=1) as p_wres,
                tc.tile_pool(name="gath_a", bufs=3) as p_gath,
                tc.tile_pool(name="aggT_a", bufs=2) as p_aggT,
            ):
                def layer_nm_pipelined(src_full, fa, fo, w_sb, brow,
                                       nm, full, src_lay, dst_lay,
                                       warm=False, pre_tail=None,
                                       pre_map=None):
                    """Block-pipelined layer: GEMM each tile as soon as it is
                    aggregated; fire the AllGather for block k right after
                    its last tile's GEMM, while aggregating 4 tiles ahead —
                    the serial AG chain starts (and ends) as early as the
                    GEMM pace allows.  pre_tail() runs just before the tail
                    AG emission (to pre-stage the next layer's gathers);
                    pre_map supplies THIS layer's hoisted gather tiles."""
                    sbidx = dst_lay[2]
                    pre_map = pre_map or {}
                    shp = [128, fa // 128, 1024]
                    aggT = [p_aggT.tile(shp, BF, tag="aggT", name="aggTq0"),
                            None]
                    for tt in range(4):
                        agg_tile(src_full, fa, tt, tt, aggT[0],
                                 src_lay=src_lay, warm=warm,
                                 pre=pre_map.get(tt))
                    for t in range(NT):
                        gemm_tile(aggT[t // 8], t // 8, t % 8, fa, fo,
                                  w_sb, brow, True, nm)
                        nt = t + 4
                        if nt < NT:
                            if nt == 8:
                                aggT[1] = p_aggT.tile(shp, BF,
                                                      tag="aggT",
                                                      name="aggTq1")
                            agg_tile(src_full, fa, nt, nt % 8,
                                     aggT[nt // 8], src_lay=src_lay,
                                     warm=warm)
                        if t + 1 in sbidx:
                            if t + 1 == NT and pre_tail is not None:
                                pre_tail()
                            allgather_rows(nm, full, sbidx[t + 1], dst_lay)

                # ---- layer 1 ----
                w_sb = p_wres.tile([128, 4, 1024], BF, tag="wres")
                nc.sync.dma_start(out=w_sb[:], in_=w1_d[:])
                brow = p_brow.tile([128, 1024], BF, tag="brow")
                nc.sync.dma_start(out=brow[:], in_=brow1_d[:])
                hoisted = {}
                layer_nm_pipelined(None, DIMS[0], DIMS[1], w_sb, brow,
                                   h2_nm, h2_full, None, LAY2, warm=True,
                                   pre_tail=lambda: hoisted.update(
                                       hoist_early(h2_full, DIMS[1], src2,
                                                   len(B_H2))))
                if dbg_d is not None:
                    nc.sync.dma_start(out=dbg_d.ap()[:], in_=h2_full.ap()[:])

                # ---- layer 2 ----
                w_sb = p_wres.tile([128, 8, 2048], BF, tag="wres")
                nc.sync.dma_start(out=w_sb[:], in_=w2_d[:])
                brow = p_brow.tile([128, 2048], BF, tag="brow")
                nc.sync.dma_start(out=brow[:], in_=brow2_d[:])
                hoisted3 = {}
                layer_nm_pipelined(h2_full, DIMS[1], DIMS[2], w_sb,
                                   brow, h3_nm, h3_full, src2, LAY3,
                                   warm=True, pre_map=hoisted,
                                   pre_tail=lambda: hoisted3.update(
                                       hoist_early(h3_full, DIMS[2], src3,
                                                   len(B_H3))))
                if dbg3_d is not None:
                    nc.sync.dma_start(out=dbg3_d.ap()[:], in_=h3_full.ap()[:])

                # ---- layer 3 (feature-major out, no AG) ----
                b3c = p_bcol.tile([128, 32], F32, tag="bcol")
                nc.sync.dma_start(out=b3c[:], in_=b3c_d[:])
                aggT0 = p_aggT.tile([128, 16, 1024], BF, tag="aggT")
                for tt in range(8):
                    agg_tile(h3_full, DIMS[2], tt, tt, aggT0, src_lay=src3,
                             pre=hoisted3.get(tt))
                aggT1 = p_aggT.tile([128, 16, 1024], BF, tag="aggT")
                for tt in range(8):
                    agg_tile(h3_full, DIMS[2], 8 + tt, tt, aggT1,
                             src_lay=src3)
                    gemm_fm3(aggT0, 0, b3c, mbs=range(tt * 4, tt * 4 + 4))
                gemm_fm3(aggT1, 1, b3c)

            # ================= layer 4 GEMM: m4 = h4 @ W4 =================
            with (
                tc.tile_pool(name="w4", bufs=4) as p_w4,
                tc.tile_pool(name="h4t", bufs=2) as p_h4t,
            ):
                # load order: the quarter tile 0 needs first, then its
                # lt, then the rest — with 512-wide per-quarter PSUM groups
                # the first matmuls start after a single 4 MB quarter lands
                # instead of waiting out the full 16 MB W4 burst
                w4q = [None] * 4
                w4q[0] = p_w4.tile([128, 32, 512], BF, tag="w4", name="w4q0")
                nc.sync.dma_start(out=w4q[0][:], in_=w4_d[:, :, 0:512])
                lt0 = p_h4t.tile([128, 32, 128], BF, tag="h4t")
                nc.sync.dma_start(out=lt0[:], in_=h4T.ap()[0, :, :, :])
                for q in (1, 2, 3):
                    w4q[q] = p_w4.tile([128, 32, 512], BF, tag="w4",
                                       name=f"w4q{q}")
                    nc.sync.dma_start(out=w4q[q][:],
                                      in_=w4_d[:, :, q * 512:(q + 1) * 512])
                if True:
                    for t in range(NT):
                        if t == 0:
                            lt = lt0
                        else:
                            lt = p_h4t.tile([128, 32, 128], BF, tag="h4t")
                            nc.sync.dma_start(out=lt[:],
                                              in_=h4T.ap()[t, :, :, :])
                        for hf in range(4):
                            gps = p_gemm.tile([128, 512], F32, tag="gemmps")
                            for kb in range(32):
                                nc.tensor.matmul(
                                    out=gps[:],
                                    lhsT=lt[:, kb, :],
                                    rhs=w4q[hf][:, kb, :],
                                    start=(kb == 0), stop=(kb == 31),
                                )
                            mt = p_ht.tile([128, 512], BF, tag="ht")
                            nc.vector.tensor_copy(out=mt[:], in_=gps[:])
                            nc.sync.dma_start(
                                out=m4_nm.ap()[t * 128:(t + 1) * 128,
                                               hf * 512:(hf + 1) * 512],
                                in_=mt[:],
                            )
                        if t + 1 in LAY4[2] and LAY4[2][t + 1] != len(B_M4) - 1:
                            allgather_rows(m4_nm, m4_full, LAY4[2][t + 1],
                                           LAY4)
                # the TAIL m4 AllGather is emitted in the head phase below,
                # after the first tiles' early gather groups: the AG blocks
                # the in-order gpsimd stream for its whole duration, so any
                # gather emitted after it waits even if its data landed long
                # ago.
                if dbg4_d is not None:
                    nc.sync.dma_start(out=dbg4_d.ap()[:], in_=m4_full.ap()[:])

            # ======== layer 4 aggregation + bias + relu, head GEMM ========
            with (
                tc.tile_pool(name="gath_b", bufs=3) as p_gath,
                tc.tile_pool(name="aggT_b", bufs=2) as p_aggT,
                tc.tile_pool(name="whead", bufs=1) as p_whead,
                tc.tile_pool(name="outf", bufs=2) as p_outf,
            ):
                wo_sb = p_whead.tile([128, 16, CPAD], BF)
                nc.sync.dma_start(out=wo_sb[:], in_=wo_d[:])
                browo = p_brow.tile([128, CPAD], BF, tag="brow")
                nc.sync.dma_start(out=browo[:], in_=browo_d[:])
                b4c = p_bcol.tile([128, 16], F32, tag="bcol")
                nc.sync.dma_start(out=b4c[:], in_=b4c_d[:])

                isb4, dept4, fullbase4 = src4
                ntail = len(B_M4) - 1

                def groups4(t):
                    nct = cnt_t[t]
                    groups = []
                    c0 = 0
                    for c in range(1, nct + 1):
                        if c == nct or dept4[t][c] != dept4[t][c0]:
                            groups.append((c0, c, dept4[t][c0]))
                            c0 = c
                    return groups

                def gather4(gath, t, gs):
                    for (c0, c1, dep) in gs:
                        ng = c1 - c0
                        rows = fullbase4[dep + 1]
                        nc.gpsimd.dma_gather(
                            gath[:, c0:c1, :], m4_full.ap()[0:rows, :],
                            isb4[:, t, c0 * 8:c1 * 8],
                            ng * 128, ng * 128, DIMS[4],
                        )

                # hoist tiles 0-1's early gather groups (data already landed)
                # ahead of the blocking tail AG so the PE has aggregation
                # work during the AG's gpsimd-stream occupancy
                hoisted = {}
                for t in (0, 1, 2):
                    gath = p_gath.tile([128, chunks, DIMS[4]], BF,
                                       tag="gath", name=f"gath4_{t}")
                    gs = groups4(t)
                    early = [g for g in gs if g[2] < ntail]
                    late = [g for g in gs if g[2] >= ntail]
                    gather4(gath, t, early)
                    hoisted[t] = (gath, late)
                allgather_rows(m4_nm, m4_full, ntail, LAY4)

                # Fused per-tile aggregation -> head GEMM: the head matmuls
                # for tile t fill the PE while tile t+1's gathers are in
                # flight (the in-order PE queue would otherwise run every
                # agg matmul before the first head matmul).
                for half in range(2):
                    aggT = p_aggT.tile([128, 16, 1024], BF, tag="aggT")
                    for tt in range(8):
                        t = half * 8 + tt
                        nct = cnt_t[t]
                        if t in hoisted:
                            gath, late = hoisted[t]
                            gather4(gath, t, late)
                        else:
                            gath = p_gath.tile([128, chunks, DIMS[4]], BF,
                                               tag="gath")
                            gather4(gath, t, groups4(t))
                        for g in range(DIMS[4] // 512):
                            aps = p_agg.tile([128, 4, 128], F32, tag="aggps")
                            for q in range(4):
                                fb = g * 4 + q
                                for c in range(nct):
                                    nc.tensor.matmul(
                                        out=aps[:, q, :],
                                        lhsT=gath[:, c,
                                                  fb * 128:(fb + 1) * 128],
                                        rhs=s_sb[:, t, c, :],
                                        start=(c == 0), stop=(c == nct - 1),
                                    )
                            for q in range(4):
                                fb = g * 4 + q
                                nc.vector.tensor_scalar(
                                    out=aggT[:, fb, tt * 128:(tt + 1) * 128],
                                    in0=aps[:, q, :],
                                    scalar1=b4c[:, fb:fb + 1], scalar2=0.0,
                                    op0=mybir.AluOpType.add,
                                    op1=mybir.AluOpType.max,
                                )
                        # head GEMM for this tile
                        gps = p_gemm.tile([128, CPAD], F32, tag="gemmps")
                        for kb in range(16):
                            for cc in range(2):
                                nc.tensor.matmul(
                                    out=gps[:, cc * 512:(cc + 1) * 512],
                                    lhsT=aggT[:, kb, tt * 128:(tt + 1) * 128],
                                    rhs=wo_sb[:, kb, cc * 512:(cc + 1) * 512],
                                    start=(kb == 0), stop=False,
                                )
                        for cc in range(2):
                            nc.tensor.matmul(
                                out=gps[:, cc * 512:(cc + 1) * 512],
                                lhsT=ones_sb[:, 0:128],
                                rhs=browo[:, cc * 512:(cc + 1) * 512],
                                start=False, stop=True,
                            )
                        of = p_outf.tile([128, CPAD], F32, tag="outf")
                        nc.vector.tensor_copy(out=of[:], in_=gps[:])
                        nc.sync.dma_start(
                            out=out_d.ap()[t * 128:(t + 1) * 128, :],
                            in_=of[:],
                        )

    nc.compile()
    return nc


# ----------------------------------------------------------------------------
# Host-side preprocessing
# ----------------------------------------------------------------------------

def _balance_tiles(wt):
    """Assign nodes to 128 tiles of exactly 128 nodes, balancing total
    weight; heaviest tiles go to the same tile POSITION on every core so
    the (core-uniform) per-position chunk counts stay minimal.

    Returns perm[new_position] = node."""
    order = np.argsort(-wt, kind="stable")
    nbins = 128
    bins = [[] for _ in range(nbins)]
    bw = np.zeros(nbins, np.int64)
    bn = np.zeros(nbins, np.int64)
    for n in order:
        open_b = bn < 128
        cand = np.where(open_b)[0]
        b = cand[np.argmin(bw[cand])]
        bins[b].append(n)
        bw[b] += wt[n]
        bn[b] += 1
    # local refinement: swap nodes between heaviest/lightest bins
    for _ in range(256):
        hi, lo = int(np.argmax(bw)), int(np.argmin(bw))
        if bw[hi] - bw[lo] <= 1:
            break
        d = bw[hi] - bw[lo]
        ah, al = np.asarray(bins[hi]), np.asarray(bins[lo])
        diff = wt[ah][:, None] - wt[al][None, :]
        good = (diff > 0) & (diff <= d)
        if not good.any():
            break
        # pick the swap closest to halving the imbalance
        score = np.where(good, -np.abs(diff - d // 2), -10**9)
        ii, jj = np.unravel_index(np.argmax(score), diff.shape)
        ni, nj = int(ah[ii]), int(al[jj])
        bins[hi][int(ii)], bins[lo][int(jj)] = nj, ni
        delta = wt[ni] - wt[nj]
        bw[hi] -= delta
        bw[lo] += delta
    # lightest bins to the two END tile positions (pipeline head starts the
    # AG chain, tail finishes right after the last block lands); heaviest in
    # the middle.  Same position on every core so the core-uniform chunk
    # counts stay minimal.
    pos_order = []
    lo, hi = 0, NT - 1
    while lo <= hi:
        pos_order.append(lo)
        if hi != lo:
            pos_order.append(hi)
        lo += 1
        hi -= 1
    bin_order = np.argsort(bw)                # light..heavy
    perm = np.zeros(N, np.int64)
    for i, b in enumerate(bin_order):
        t = pos_order[i // NCORES]             # tile position
        r = i % NCORES                         # core
        g = r * NT + t
        perm[g * 128:(g + 1) * 128] = bins[b]
    return perm


def _remap(n, blocks):
    """AG'd tensors land as row blocks: block k holds rank r's shard rows
    [SBT[k]*128, SBT[k+1]*128) at full rows FULLBASE[k] + r*blocks[k]*128."""
    sbt, fullbase, _ = _layout(blocks)
    blk_of = np.repeat(np.arange(len(blocks)), blocks)
    r = n // NL
    l = n % NL
    k = blk_of[l // 128]
    fb = np.asarray(fullbase)[k]
    sb = np.asarray(sbt)[k] * 128
    bw = np.asarray(blocks)[k] * 128
    return (fb + r * bw + (l - sb)).astype(np.int64)


def _prep_graph(edge_src, edge_dst, edge_weight):
    src = np.asarray(edge_src).astype(np.int64)
    dst = np.asarray(edge_dst).astype(np.int64)
    ew = np.asarray(edge_weight).astype(np.float64)

    deg = np.bincount(dst, weights=ew, minlength=N) + 1.0
    dinv = 1.0 / np.sqrt(deg)
    norm = (dinv[src] * ew * dinv[dst]).astype(np.float32)
    selfc = (dinv * dinv).astype(np.float32)

    # balance in-degree(+self) across tiles with a global permutation:
    # position p holds node perm[p]; ipos[node] = position
    wt = np.bincount(dst, minlength=N).astype(np.int64) + 1
    perm = _balance_tiles(wt)
    ipos = np.zeros(N, np.int64)
    ipos[perm] = np.arange(N)

    # combined edge + self-loop lists, in position space
    nodes = np.arange(N)
    asrc = ipos[np.concatenate([src, nodes])]
    adst = ipos[np.concatenate([dst, nodes])]
    aval = np.concatenate([norm, selfc])

    gtile = adst // 128                     # global dst tile 0..127
    counts = np.bincount(gtile, minlength=128)
    # per tile POSITION (max over cores) chunk count, core-uniform program
    cnt_rt = counts.reshape(NCORES, NT)
    cnt_t = tuple(int(np.ceil(cnt_rt[:, t].max() / 128.0)) for t in range(NT))
    chunks = max(cnt_t)
    cap = chunks * 128

    # sort slots within each tile by source (tile position, core, offset) --
    # a layout-neutral key so one sort order staircases the AG-block deps of
    # ALL three h_full layouts (block boundaries are tile-position intervals)
    skey = ((asrc % NL) // 128) * (NCORES * 128) \
        + (asrc // NL) * 128 + (asrc % 128)
    order = np.lexsort((skey, gtile))
    starts = np.zeros(128, np.int64)
    starts[1:] = np.cumsum(counts)[:-1]
    pos_sorted = np.arange(len(asrc)) - starts[gtile[order]]

    val_all = np.zeros((128, cap), np.float32)
    m_all = np.zeros((128, cap), np.int64)
    src_all = np.full((128, cap), -1, np.int64)     # position-space source
    val_all[gtile[order], pos_sorted] = aval[order]
    m_all[gtile[order], pos_sorted] = adst[order] - gtile[order] * 128
    src_all[gtile[order], pos_sorted] = asrc[order]

    # per-layout remapped indices + AG-block deps per (tile pos, chunk)
    pad = src_all < 0
    idx_alls, dep_ts = [], []
    for blocks in (B_H2, B_H3, B_M4):
        rmp = _remap(np.where(pad, 0, src_all).reshape(-1), blocks)
        rmp = np.where(pad.reshape(-1), 0, rmp).reshape(128, cap)
        idx_alls.append(rmp.astype(np.int16))
        fb = np.asarray(_layout(blocks)[1])
        dep_t = []
        for t in range(NT):
            deps = []
            for c in range(cnt_t[t]):
                hi = rmp.reshape(NCORES, NT, cap)[:, t,
                                                  c * 128:(c + 1) * 128]
                deps.append(int(np.searchsorted(fb, hi.max(), side="right")
                                - 1))
            dep_t.append(tuple(deps))
        dep_ts.append(tuple(dep_t))

    # dense scatter matrices S[tile, chunk, k, m]
    s_dense = np.zeros((128, chunks, 128, 128), np.float32)
    ttg = np.repeat(np.arange(128), cap)
    pp = np.tile(np.arange(cap), 128)
    s_dense[ttg, pp // 128, pp % 128, m_all.reshape(-1)] = val_all.reshape(-1)
    # padding slots (val 0) may alias dst 0; they contribute 0 regardless.

    # per-core device layouts
    idx_dev = np.zeros((3, NCORES, 128, NT, chunks * 8), np.int16)
    s_dev = np.zeros((NCORES, 128, NT, chunks, 128), NPBF)
    for r in range(NCORES):
        for t in range(NT):
            g = r * NT + t
            for li in range(3):
                packed = idx_alls[li][g].reshape(-1, 16).T  # [16, chunks*8]
                idx_dev[li, r, :, t, :] = np.tile(packed, (8, 1))
            s_dev[r, :, t, :, :] = s_dense[g].transpose(1, 0, 2).astype(NPBF)
    return chunks, cnt_t, tuple(dep_ts), perm, idx_dev, s_dev


def _prep_weights(inputs):
    """Natural (feature-major-contraction) weight layouts."""
    W1 = np.asarray(inputs["W1"], np.float32)
    W2 = np.asarray(inputs["W2"], np.float32)
    W3 = np.asarray(inputs["W3"], np.float32)
    W4 = np.asarray(inputs["W4"], np.float32)
    Wo = np.zeros((DIMS[4], CPAD), np.float32)
    Wo[:, :C] = np.asarray(inputs["Wout"], np.float32)

    def nat(w):   # [fa, fo] -> [128, fa//128, fo]
        fa, fo = w.shape
        return np.ascontiguousarray(
            w.reshape(fa // 128, 128, fo).transpose(1, 0, 2)).astype(NPBF)

    # W3 as lhsT blocks [128, mb, kb, 128]
    w3b = np.ascontiguousarray(
        W3.reshape(16, 128, 32, 128).transpose(1, 2, 0, 3)).astype(NPBF)

    b1 = np.asarray(inputs["b1"], np.float32)
    b2 = np.asarray(inputs["b2"], np.float32)
    b3 = np.asarray(inputs["b3"], np.float32)
    b4 = np.asarray(inputs["b4"], np.float32)
    bo = np.zeros(CPAD, np.float32)
    bo[:C] = np.asarray(inputs["bout"], np.float32)

    def brow(b, n):
        r = np.zeros((128, n), NPBF)
        r[0, :] = b.astype(NPBF)
        return r

    ones = np.zeros((128, 512), NPBF)
    ones[0, :128] = NPBF(1.0)

    return {
        "w1n": nat(W1), "w2n": nat(W2), "w3b": w3b, "w4n": nat(W4),
        "won": nat(Wo),
        "brow1": brow(b1, 1024), "brow2": brow(b2, 2048),
        "browo": brow(bo, CPAD),
        "b3c": np.ascontiguousarray(b3.reshape(32, 128).T),
        "b4c": np.ascontiguousarray(b4.reshape(16, 128).T),
        "onesrow": ones,
    }


def _run(inputs, trace=False, **kw):
    x = np.asarray(inputs["x"], np.float32)
    chunks, cnt_t, dep_t, perm, idx_dev, s_dev = _prep_graph(
        inputs["edge_src"], inputs["edge_dst"], inputs["edge_weight"])
    wmap = _prep_weights(inputs)

    key = (chunks, cnt_t, dep_t)
    if key not in _CACHE:
        _CACHE[key] = _build(chunks, cnt_t, *dep_t)
    nc = _CACHE[key]

    # position p holds node perm[p]; x rows land at the B_H2 block-split AG
    # remap of p, then the layer-1 gather is applied on the host (with the
    # B_H2-layout idx): the device streams contiguous pre-gathered tiles
    # instead of running Q7 gather descriptor generation on the critical path
    rmp = _remap(np.arange(N), B_H2)
    x_rm = np.empty_like(x)
    x_rm[rmp] = x[perm]
    x_bf = np.ascontiguousarray(x_rm).astype(NPBF)
    in_maps = []
    for r in range(NCORES):
        # idx_dev packs slot s at [s % 16, s // 16], tiled 8x over 128
        idx16 = idx_dev[0, r][:16]               # [16, NT, chunks*8]
        slots = idx16.transpose(1, 2, 0).reshape(NT, -1)   # [NT, cap]
        g = x_bf[slots.astype(np.int64)]         # [NT, cap, F]
        xg = np.ascontiguousarray(
            g.reshape(NT, chunks, 128, DIMS[0]).transpose(2, 0, 1, 3))
        m = {"xg": xg, "idx2": idx_dev[0, r], "idx3": idx_dev[1, r],
             "idx4": idx_dev[2, r], "smat": s_dev[r], **wmap}
        in_maps.append(m)

    res = run_bass_kernel_spmd(nc, in_maps, core_ids=list(range(NCORES)),
                               trace=trace, **kw)
    dev = np.concatenate(
        [res.results[r]["out_nm"][:, :C] for r in range(NCORES)], axis=0)
    out = np.empty_like(dev)
    out[perm] = dev
    return np.ascontiguousarray(out.astype(np.float32)), res


def kernel(**inputs) -> np.ndarray:
    out, _ = _run(inputs, trace=False)
    return out

